# revision 14
# baseline (speedup 1.0000x reference)
"""Trainium2 Bass kernel for nn_DistributionLossWithLabel_v2.

loss = sum_i (kl_div[i] + sum_j kl_dis[i,j]*L[i,j]) / (sum_j kl_dis[i,j]*(1-L[i,j]))

with kl_dis[i,j] = (pe[j] - logq[i]@p[j]) / D,  pe[j] = sum_d p[j,d]*log p[j,d],
kl_div[i] = (pe[i] - p[i]@logq[i]) / D.

Sharding: rows i across 8 cores (512 rows each), p replicated.
Per-core math avoids the [512,4096] pairwise block entirely:
  rs1[i]    = sum_j L[i,j]*(pe[j] - logq[i]@p[j]) = Lpe[i] - sum_d logq[i,d]*(L@p)[i,d]
  rs_all[i] = sum_j (pe[j] - logq[i]@p[j])        = SPE    - logq[i]@s      (s = colsum p)
Main GEMM (contract over j, p natural layout, bf16):
  A = [p | plogp128]^T @ L^T   where plogp128 = p*logp tree-reduced to 128 cols
  (so Lpe falls out of the plogp128 block summed over d'' - done by the 2nd GEMM
   with -1 rows; the appended ones-column MMs give the s / SPE columns.)
Second GEMM against gT = [logq^T ; -1] (fp32):  diag -> -rs1, s-column -> -rs_all.
Outputs per core: num[i] = pe_own-dotp+rs1, den[i] = rs_all-rs1; host divides in
f64 and sums (the 1/D factors cancel in the ratio).

HW notes (measured on this runtime): tensor_tensor_reduce and gpsimd dtype-cast
copies crash the device - avoided.  dma_start_transpose (bf16) is used for the
L -> W transposes; logq transposes go through the PE with identity.
"""

import numpy as np

B, D = 4096, 1024
NCORES = 8
S = B // NCORES          # 512 shard rows per core
P = 128
JT = B // P              # 32 j-tiles (p rows)
DBLK = D // P            # 8 d-blocks
IB = S // P              # 4 i-blocks per core
PLB = 1                  # plogp reduced to PLB*128 columns (3-level tree)
NBLK = DBLK + PLB        # kxm blocks in the main GEMM

_CACHE = {}

LAST_RESULTS = None      # set by kernel(); test.py reads exec_time/profile


def _build_nc():
    from contextlib import ExitStack
    import concourse.bass as bass
    import concourse.tile as tile
    import concourse.mybir as mybir
    from concourse import bacc
    from concourse.masks import make_identity

    fp32 = mybir.dt.float32
    bf16 = mybir.dt.bfloat16
    FT = mybir.ActivationFunctionType
    OP = mybir.AluOpType
    AX = mybir.AxisListType

    nc = bacc.Bacc("TRN2", target_bir_lowering=False, debug=False)
    q_d = nc.declare_dram_parameter("q", [S, D], fp32, isOutput=False)
    p_d = nc.declare_dram_parameter("p", [B, D], fp32, isOutput=False)
    po_d = nc.declare_dram_parameter("p_own", [S, D], fp32, isOutput=False)
    lab_d = nc.declare_dram_parameter("lab", [S, B], fp32, isOutput=False)
    num_d = nc.declare_dram_parameter("num", [P, IB], fp32, isOutput=True)
    den_d = nc.declare_dram_parameter("den", [P, IB], fp32, isOutput=True)

    PPW = D + PLB * P    # pp tile width (p cols + plogp cols)

    with tile.TileContext(nc) as tc, ExitStack() as ctx:
        const = ctx.enter_context(tc.tile_pool(name="const", bufs=1))
        persist = ctx.enter_context(tc.tile_pool(name="persist", bufs=1))
        trans = ctx.enter_context(tc.tile_pool(name="trans", bufs=2))
        ptrans = ctx.enter_context(tc.tile_pool(name="ptrans", bufs=3))

        # ---- constants ----
        ident = const.tile([P, P], fp32, tag="ident")
        make_identity(nc, ident[:])
        ones_col = const.tile([P, 1], bf16, tag="ones")
        nc.gpsimd.memset(ones_col[:], 1.0)
        negones = const.tile([P, P], fp32, tag="negones")
        nc.gpsimd.memset(negones[:], -1.0)

        # ---- persistent SBUF ----
        gT = [persist.tile([P, S], fp32, tag=f"gT{k}", name=f"gT{k}")
              for k in range(DBLK)]
        W = persist.tile([P, JT * S], bf16, tag="W")     # W[:, j*S+c] = L[i_c, j*P+jj]
        pp = [persist.tile([P, PPW], bf16, tag=f"pp{j}", name=f"pp{j}")
              for j in range(JT)]
        scol_sb = persist.tile([P, 16], fp32, tag="scol")
        pe_own = persist.tile([P, IB], fp32, tag="pe_own")
        dotp = persist.tile([P, IB], fp32, tag="dotp")
        diag = persist.tile([P, IB], fp32, tag="diag")
        o2s = persist.tile([P, IB], fp32, tag="o2s")
        num_sb = persist.tile([P, IB], fp32, tag="num_sb")
        den_sb = persist.tile([P, IB], fp32, tag="den_sb")
        t1 = persist.tile([P, IB], fp32, tag="t1")

        with tc.tile_pool(name="lq", bufs=1) as lq_pool:
            logq = [lq_pool.tile([P, D], fp32, tag=f"logq{b}", name=f"logq{b}")
                    for b in range(IB)]

            with tc.tile_pool(name="tp_psum", bufs=2, space="PSUM") as tp_pool:
                # ---- q: load + log + PE-transpose to gT ----
                for b in range(IB):
                    q_t = trans.tile([P, D], fp32, tag="ld")
                    nc.sync.dma_start(q_t[:], q_d[b * P:(b + 1) * P, :])
                    nc.scalar.activation(logq[b][:], q_t[:], FT.Ln)
                for k in range(DBLK):
                    pt = tp_pool.tile([P, S], fp32, tag="tp")
                    for b in range(IB):
                        nc.tensor.transpose(
                            pt[:, b * P:(b + 1) * P],
                            logq[b][:, k * P:(k + 1) * P], ident[:])
                    nc.any.tensor_copy(gT[k][:], pt[:])

                # ---- p_own: pe_own and dotp (mult on DVE, reduce on ACT) ----
                for b in range(IB):
                    po_t = trans.tile([P, D], fp32, tag="ld")
                    nc.sync.dma_start(po_t[:], po_d[b * P:(b + 1) * P, :])
                    logpo_t = trans.tile([P, D], fp32, tag="lgo")
                    nc.scalar.activation(logpo_t[:], po_t[:], FT.Ln)
                    m1 = trans.tile([P, D], fp32, tag="pom")
                    nc.vector.tensor_mul(m1[:], po_t[:], logpo_t[:])
                    s1 = trans.tile([P, D], bf16, tag="poscr")
                    nc.scalar.activation(s1[:], m1[:], FT.Copy,
                                         accum_out=pe_own[:, b:b + 1])
                    m2 = trans.tile([P, D], fp32, tag="pom")
                    nc.vector.tensor_mul(m2[:], po_t[:], logq[b][:])
                    s2 = trans.tile([P, D], bf16, tag="poscr")
                    nc.scalar.activation(s2[:], m2[:], FT.Copy,
                                         accum_out=dotp[:, b:b + 1])

                # ---- interleaved: p tiles (pp + plogp128) and L quarters (-> W)
                lsteps = [(it, jq) for jq in range(4) for it in range(IB)]
                for step in range(JT):
                    # p tile: cast to bf16, p*logp, 3-level tree to 128 cols
                    p_t = ptrans.tile([P, D], fp32, tag="p_t")
                    nc.sync.dma_start(p_t[:], p_d[step * P:(step + 1) * P, :])
                    logp_t = trans.tile([P, D], bf16, tag="logp")
                    nc.scalar.activation(logp_t[:], p_t[:], FT.Ln)
                    nc.any.tensor_copy(pp[step][:, 0:D], p_t[:])
                    pl = trans.tile([P, D], bf16, tag="pl")
                    nc.vector.tensor_mul(pl[:], pp[step][:, 0:D], logp_t[:])
                    tr1 = trans.tile([P, D // 2], bf16, tag="tr1")
                    nc.vector.tensor_add(tr1[:], pl[:, 0:512], pl[:, 512:1024])
                    tr2 = trans.tile([P, D // 4], bf16, tag="tr2")
                    nc.vector.tensor_add(tr2[:], tr1[:, 0:256], tr1[:, 256:512])
                    nc.vector.tensor_add(pp[step][:, D:D + P],
                                         tr2[:, 0:128], tr2[:, 128:256])
                    # L quarter: cast to bf16, xbar-DMA transpose into W
                    if step < 16:
                        it, jq = lsteps[step]
                        l_t = trans.tile([P, D], fp32, tag="l_t")
                        nc.sync.dma_start(
                            l_t[:],
                            lab_d[it * P:(it + 1) * P, jq * 1024:(jq + 1) * 1024])
                        lb = trans.tile([P, D], bf16, tag="lb")
                        nc.any.tensor_copy(lb[:], l_t[:])
                        for a in range(8):
                            jb = jq * 8 + a
                            nc.sync.dma_start_transpose(
                                W[:, jb * S + it * P: jb * S + (it + 1) * P],
                                lb[:, a * P:(a + 1) * P])

        # ---- main GEMM: A[k] = pp_k^T @ W_j over j; ones-column -> scol ----
        with tc.tile_pool(name="A_sb_pool", bufs=1) as A_pool:
            A_sb = [A_pool.tile([P, S], fp32, tag=f"A{k}", name=f"A{k}")
                    for k in range(NBLK)]
            with tc.tile_pool(name="mm_psum", bufs=7, space="PSUM") as mm_pool, \
                 tc.tile_pool(name="sc_psum", bufs=1, space="PSUM") as sc_pool:
                scol_ps = sc_pool.tile([P, 16], fp32, tag="scol_ps")
                A_ps = [mm_pool.tile([P, S], fp32, tag="A_ps", name=f"A_ps{k}")
                        for k in range(7)]
                for j in range(JT):
                    st = j == 0
                    sp = j == JT - 1
                    for k in range(7):
                        lhsT = pp[j][:, k * P:(k + 1) * P]
                        nc.tensor.matmul(A_ps[k][:], lhsT,
                                         W[:, j * S:(j + 1) * S],
                                         start=st, stop=sp)
                        nc.tensor.matmul(scol_ps[:, k:k + 1], lhsT, ones_col[:],
                                         start=(st and k == 0), stop=False)
                for k in range(7):
                    nc.any.tensor_copy(A_sb[k][:], A_ps[k][:])
                # sweep 2: remaining kxm blocks (7 .. NBLK-1) reuse freed banks
                A_ps2 = [mm_pool.tile([P, S], fp32, tag="A_ps", name=f"A_ps2{k}")
                         for k in range(7, NBLK)]
                for j in range(JT):
                    st = j == 0
                    sp = j == JT - 1
                    for k in range(7, NBLK):
                        lhsT = pp[j][:, k * P:(k + 1) * P]
                        nc.tensor.matmul(A_ps2[k - 7][:], lhsT,
                                         W[:, j * S:(j + 1) * S],
                                         start=st, stop=sp)
                        # scol group stop on the very last scol matmul
                        nc.tensor.matmul(scol_ps[:, k:k + 1], lhsT, ones_col[:],
                                         start=False,
                                         stop=(sp and k == NBLK - 1))
                for k in range(7, NBLK):
                    nc.any.tensor_copy(A_sb[k][:], A_ps2[k - 7][:])
                nc.any.tensor_copy(scol_sb[:, 0:NBLK], scol_ps[:, 0:NBLK])

            # ---- second GEMM: out2 = gT^T @ [A | scol] ----
            with tc.tile_pool(name="o2_psum", bufs=2, space="PSUM") as o2_pool:
                for b in range(IB):
                    o2 = o2_pool.tile([P, P], fp32, tag="o2")
                    o2c = o2_pool.tile([P, 1], fp32, tag="o2c")
                    for k in range(NBLK):
                        lhsT = (gT[k][:, b * P:(b + 1) * P] if k < DBLK
                                else negones[:])
                        nc.tensor.matmul(o2[:], lhsT,
                                         A_sb[k][:, b * P:(b + 1) * P],
                                         start=(k == 0), stop=(k == NBLK - 1))
                        nc.tensor.matmul(o2c[:], lhsT, scol_sb[:, k:k + 1],
                                         start=(k == 0), stop=(k == NBLK - 1))
                    scr = trans.tile([P, P], fp32, tag="scr_o2")
                    nc.vector.tensor_mul(scr[:], o2[:], ident[:])
                    nc.vector.tensor_reduce(out=diag[:, b:b + 1], in_=scr[:],
                                            axis=AX.X, op=OP.add)
                    nc.any.tensor_copy(o2s[:, b:b + 1], o2c[:])

        # ---- finals: num = pe_own - dotp - diag ; den = diag - o2s ----
        nc.vector.tensor_sub(t1[:], pe_own[:], dotp[:])
        nc.vector.tensor_sub(num_sb[:], t1[:], diag[:])
        nc.vector.tensor_sub(den_sb[:], diag[:], o2s[:])
        nc.sync.dma_start(num_d[:, :], num_sb[:])
        nc.sync.dma_start(den_d[:, :], den_sb[:])

    nc.compile()
    return nc


def kernel(q, p, labels_matrix):
    global LAST_RESULTS
    from concourse.bass_utils import run_bass_kernel_spmd

    if "nc" not in _CACHE:
        _CACHE["nc"] = _build_nc()
    nc = _CACHE["nc"]

    q = np.ascontiguousarray(np.asarray(q, dtype=np.float32))
    p = np.ascontiguousarray(np.asarray(p, dtype=np.float32))
    lab = np.ascontiguousarray(np.asarray(labels_matrix, dtype=np.float32))

    in_maps = []
    for c in range(NCORES):
        in_maps.append({
            "q": np.ascontiguousarray(q[c * S:(c + 1) * S]),
            "p": p,
            "p_own": np.ascontiguousarray(p[c * S:(c + 1) * S]),
            "lab": np.ascontiguousarray(lab[c * S:(c + 1) * S]),
        })

    res = run_bass_kernel_spmd(nc, in_maps, list(range(NCORES)))
    LAST_RESULTS = res

    total = 0.0
    for c in range(NCORES):
        num = np.asarray(res.results[c]["num"]).T.ravel().astype(np.float64)
        den = np.asarray(res.results[c]["den"]).T.ravel().astype(np.float64)
        total += float(np.sum(num / den))
    return np.float32(total)
